# revision 11
# baseline (speedup 1.0000x reference)
"""Distributed Trainium2 kernel for a single attention head.

Problem: x:[8,2048,1024] f32, w_q/w_k/w_v:[1024,64] f32
  q,k,v = x@w ; scores = (q k^T)/sqrt(1024) causal-masked; out = softmax(scores)@v

Sharding: data-parallel over batch B=8 across the 8 NeuronCores (one batch
element per core, weights replicated, no collectives).

Per-core dataflow (T=2048, C=1024, H=64):
  - host ships x^T c-tiled [128, 8, T] bf16 and packed wqkv [128, 8, 192]
    bf16; output is written tiled [128, 16, 64] f32 and re-permuted on host.
  - projections: weights stationary, K=128 c-tiles (PSUM accumulation caps
    at 128 contraction elems/cycle, so row-packing buys nothing here);
    qk packed in one pass (M=128), v in a second (M=64).
  - scores per s-tile pair run concurrently in PE row groups 0/1 (single-
    shot K=64 matmuls reach 2 cols/cycle); q/k duplicated onto both
    partition halves via SBUF DMA to enable the packing.
  - causal diag blocks: -1e5 additive mask via identity matmul on the PE
    (keeps the scores->exp->PV chain on two engines only).
  - exp on ScalarE with scale=1/32 folded in, fully-masked column prefix
    trimmed; output bf16.
  - PV: out^T[h,t] accumulated serially over s-tiles with lhsT = [v | 1] so
    row 64 is the softmax denominator (fused row-sum).
  - epilogue: f32 PE transpose back to [t,h], reciprocal-scale on VectorE,
    one tiled output DMA per chunk; acc is double-buffered so the next
    chunk's PV does not wait on the epilogue copy.
  - x input: chunk 0 c-tiles split across the 3 DMA queues for latency;
    chunks 1-3 as half-chunk strided DMAs; next-chunk projection emission
    interleaved between score pairs keeps the PE queue dense.
"""

import os
import sys

import numpy as np

for p in ("/opt/trn_rl_repo",):
    if p not in sys.path and os.path.isdir(p):
        sys.path.insert(0, p)

import ml_dtypes  # noqa: E402

B, T, C, H = 8, 2048, 1024, 64
N_CORES = 8
TCH = 512                  # t-chunk (columns per PSUM bank of f32)
N_CHUNK = T // TCH         # 4
N_CT = C // 128            # 8 contraction tiles
SCALE = float(C) ** -0.5   # 1/32

_CACHE = {}


def _build():
    """Build + compile the SPMD Bass graph (same graph on all 8 cores)."""
    import concourse.bass as bass
    import concourse.mybir as mybir
    import concourse.tile as tile
    from concourse import bacc

    f32 = mybir.dt.float32
    bf16 = mybir.dt.bfloat16
    EXP = mybir.ActivationFunctionType.Exp

    nc = bacc.Bacc(
        "TRN2", target_bir_lowering=False, debug=False, num_devices=N_CORES
    )

    xT_d = nc.dram_tensor("xTt", [128, N_CT, T], bf16, kind="ExternalInput")
    wqkv_d = nc.dram_tensor("wqkv", [128, N_CT, 192], bf16, kind="ExternalInput")
    mask_d = nc.dram_tensor("maskb", [128, 128], bf16, kind="ExternalInput")
    idb_d = nc.dram_tensor("idb", [128, 128], bf16, kind="ExternalInput")
    idf_d = nc.dram_tensor("idf", [128, 128], f32, kind="ExternalInput")
    out_d = nc.dram_tensor("outt", [128, T // 128, H], f32, kind="ExternalOutput")

    with tile.TileContext(nc) as tc:
        with (
            tc.tile_pool(name="const", bufs=1) as constp,
            tc.tile_pool(name="xTp", bufs=1) as xTp,
            tc.tile_pool(name="qkp", bufs=1) as qkp,
            tc.tile_pool(name="v1p", bufs=1) as v1p,
            tc.tile_pool(name="exp", bufs=6) as expp,
            tc.tile_pool(name="epi", bufs=2) as epip,
            tc.tile_pool(name="outb", bufs=2) as outp,
            tc.tile_pool(name="Sp", bufs=2, space="PSUM") as Sp,
            tc.tile_pool(name="projp", bufs=2, space="PSUM") as projp,
            tc.tile_pool(name="accp", bufs=2, space="PSUM") as accp,
        ):
            # ---- weights: c-tile 0 first (unblocks first matmul)
            wqkv_t = constp.tile([128, N_CT, 192], bf16, tag="wqkv", name="wqkv_t")
            nc.sync.dma_start(out=wqkv_t[:, 0, :], in_=wqkv_d[:, 0, :])

            mask_t = constp.tile([128, 128], bf16, tag="mask", name="mask_t")
            nc.gpsimd.dma_start(out=mask_t[:], in_=mask_d[:])
            idb_t = constp.tile([128, 128], bf16, tag="idb", name="idb_t")
            nc.gpsimd.dma_start(out=idb_t[:], in_=idb_d[:])
            idf_t = constp.tile([128, 128], f32, tag="idf", name="idf_t")
            nc.gpsimd.dma_start(out=idf_t[:], in_=idf_d[:])

            # ---- x^T input feed (3 DMA queues: sync / scalar / gpsimd)
            xt = {}
            for t in range(N_CHUNK):
                xt[t] = xTp.tile([128, N_CT, TCH], bf16, tag=f"x{t}", name=f"x{t}")
            nc.sync.dma_start(out=xt[0][:, 0:1, :], in_=xT_d[:, 0:1, 0:TCH])
            nc.scalar.dma_start(out=xt[0][:, 1:3, :], in_=xT_d[:, 1:3, 0:TCH])
            nc.gpsimd.dma_start(out=xt[0][:, 3:5, :], in_=xT_d[:, 3:5, 0:TCH])
            nc.sync.dma_start(out=xt[0][:, 5:8, :], in_=xT_d[:, 5:8, 0:TCH])
            nc.scalar.dma_start(out=wqkv_t[:, 1:, :], in_=wqkv_d[:, 1:, :])
            nc.gpsimd.dma_start(
                out=xt[1][:, 0:4, :], in_=xT_d[:, 0:4, TCH : 2 * TCH]
            )
            nc.scalar.dma_start(
                out=xt[1][:, 4:8, :], in_=xT_d[:, 4:8, TCH : 2 * TCH]
            )
            nc.scalar.dma_start(
                out=xt[2][:, 0:4, :], in_=xT_d[:, 0:4, 2 * TCH : 3 * TCH]
            )
            nc.gpsimd.dma_start(
                out=xt[2][:, 4:8, :], in_=xT_d[:, 4:8, 2 * TCH : 3 * TCH]
            )
            nc.gpsimd.dma_start(out=xt[3][:, 0:4, :], in_=xT_d[:, 0:4, 3 * TCH :])
            nc.scalar.dma_start(out=xt[3][:, 4:8, :], in_=xT_d[:, 4:8, 3 * TCH :])

            q2 = {}   # [128, TCH] bf16: qT duplicated on both partition halves
            k2 = {}   # [128, TCH] bf16: kT duplicated on both partition halves
            v1 = {}   # [128, 65] bf16 per s-tile: [v | 1]

            def proj_steps(tch):
                """Emission thunks for chunk `tch`'s projections."""
                qk_steps = []
                v_steps = []
                st = {}

                def qk_mm(c):
                    def f():
                        if c == 0:
                            st["S"] = projp.tile(
                                [128, TCH], f32, tag="pj", name=f"Sqk{tch}"
                            )
                        nc.tensor.matmul(
                            st["S"][:],
                            wqkv_t[:, c, 0:128],
                            xt[tch][:, c, :],
                            start=(c == 0),
                            stop=(c == N_CT - 1),
                            skip_group_check=True,
                        )
                    return f

                def qk_out():
                    qt = qkp.tile([128, TCH], bf16, tag=f"q2_{tch}", name=f"q2_{tch}")
                    kt = qkp.tile([128, TCH], bf16, tag=f"k2_{tch}", name=f"k2_{tch}")
                    nc.vector.tensor_copy(qt[0:64, :], st["S"][0:64, :])
                    nc.vector.tensor_copy(kt[64:128, :], st["S"][64:128, :])
                    nc.sync.dma_start(out=qt[64:128, :], in_=qt[0:64, :])
                    nc.sync.dma_start(out=kt[0:64, :], in_=kt[64:128, :])
                    q2[tch] = qt
                    k2[tch] = kt

                def v_mm(c):
                    def f():
                        if c == 0:
                            st["Pv"] = projp.tile(
                                [64, TCH], f32, tag="pj", name=f"Pv{tch}"
                            )
                        nc.tensor.matmul(
                            st["Pv"][:],
                            wqkv_t[:, c, 128:192],
                            xt[tch][:, c, :],
                            start=(c == 0),
                            stop=(c == N_CT - 1),
                            skip_group_check=True,
                        )
                    return f

                def v_out():
                    vTt = qkp.tile([64, TCH], f32, tag=f"vT{tch}", name=f"vT{tch}")
                    nc.vector.tensor_copy(vTt[:], st["Pv"][:])
                    st["vT"] = vTt

                def v1_build(i):
                    def f():
                        j = 4 * tch + i
                        Pt = projp.tile([128, 64], f32, tag="pj", name=f"Pt{j}")
                        nc.tensor.transpose(
                            Pt[:],
                            st["vT"][:, 128 * i : 128 * (i + 1)],
                            idf_t[0:64, 0:64],
                        )
                        v1t = v1p.tile([128, 65], bf16, tag=f"v1_{j}", name=f"v1_{j}")
                        nc.vector.tensor_copy(v1t[:, 0:64], Pt[:])
                        nc.vector.memset(v1t[:, 64:65], 1.0)
                        v1[j] = v1t
                    return f

                for c in range(N_CT):
                    qk_steps.append(qk_mm(c))
                qk_steps.append(qk_out)
                for c in range(N_CT):
                    v_steps.append(v_mm(c))
                v_steps.append(v_out)
                for i in range(4):
                    v_steps.append(v1_build(i))
                return qk_steps, v_steps

            # chunk-0 qk projection up front (v after the first scores pair)
            qk0, v0 = proj_steps(0)
            for s in qk0:
                s()

            for tch in range(N_CHUNK):
                if tch + 1 < N_CHUNK:
                    qkn, vn = proj_steps(tch + 1)
                    pending = qkn + vn
                else:
                    pending = []
                jmax = 4 * tch + 3
                pairs = list(range(0, jmax + 1, 2))
                per_pair = -(-len(pending) // len(pairs)) if pending else 0

                acc = accp.tile([65, TCH], f32, tag="acc", name=f"acc{tch}")
                for pi, jp in enumerate(pairs):
                    S2 = Sp.tile([128, 2, TCH], f32, tag="S", name=f"S{tch}_{jp}")
                    for jj in range(2):
                        j = jp + jj
                        half = slice(64 * jj, 64 * (jj + 1))
                        ksl = k2[j // 4][half, 128 * (j % 4) : 128 * (j % 4 + 1)]
                        rel = j - 4 * tch
                        nc.tensor.matmul(
                            S2[:, jj, :],
                            ksl,
                            q2[tch][half, :],
                            start=True,
                            stop=(rel < 0),
                            skip_group_check=True,
                        )
                        if rel >= 0:
                            # additive -1e5 strict-lower-tri mask into the
                            # diagonal block via identity matmul (PE-only
                            # chain keeps exp off the vector engine)
                            a = 128 * rel
                            nc.tensor.matmul(
                                S2[:, jj, a : a + 128],
                                idb_t[:],
                                mask_t[:],
                                start=False,
                                stop=True,
                                skip_group_check=True,
                            )
                    # exp; skip the fully-masked column prefix of the pair
                    lo0 = 128 * max(0, jp - 4 * tch)
                    ext = expp.tile(
                        [128, 2, TCH], bf16, tag="ex", name=f"ex{tch}_{jp}"
                    )
                    nc.scalar.activation(
                        ext[:, :, lo0:], S2[:, :, lo0:], EXP, scale=SCALE
                    )
                    if tch == 0 and pi == 0:
                        for s in v0:
                            s()
                    # PV accumulation (ones col adds the softmax denominator)
                    for jj in range(2):
                        j = jp + jj
                        lo = 128 * max(0, j - 4 * tch)
                        nc.tensor.matmul(
                            acc[:, lo:TCH] if j > 0 else acc[:, :],
                            v1[j][:],
                            ext[:, jj, lo:TCH],
                            start=(j == 0),
                            stop=(j == jmax),
                            skip_group_check=True,
                        )
                    for _ in range(per_pair):
                        if pending:
                            pending.pop(0)()
                for s in pending:
                    s()

                # ======== epilogue: normalize + transpose + DMA out ========
                oT = epip.tile([65, TCH], f32, tag="oT", name=f"oT{tch}")
                nc.vector.tensor_copy(oT[:], acc[:])
                ob = outp.tile([128, 4, H], f32, tag="ob", name=f"ob{tch}")
                for i in range(4):
                    Pe = projp.tile([128, 65], f32, tag="pj", name=f"Pe{tch}_{i}")
                    nc.tensor.transpose(
                        Pe[:],
                        oT[:, 128 * i : 128 * (i + 1)],
                        idf_t[0:65, 0:65],
                    )
                    rec = epip.tile([128, 1], f32, tag="rec", name=f"rec{tch}_{i}")
                    nc.vector.reciprocal(rec[:], Pe[:, 64:65])
                    nc.vector.tensor_scalar_mul(ob[:, i, :], Pe[:, 0:64], rec[:])
                nc.sync.dma_start(
                    out=out_d[:, 4 * tch : 4 * tch + 4, :], in_=ob[:]
                )

    nc.compile()
    return nc


def _get_nc():
    if "nc" not in _CACHE:
        _CACHE["nc"] = _build()
    return _CACHE["nc"]


def _host_inputs(x, w_q, w_k, w_v):
    bf = ml_dtypes.bfloat16
    x = np.asarray(x, dtype=np.float32)
    wqkv = np.concatenate(
        [np.asarray(w_q, np.float32), np.asarray(w_k, np.float32),
         np.asarray(w_v, np.float32)], 1
    )
    wqkv_tiled = np.ascontiguousarray(
        wqkv.reshape(N_CT, 128, 192).transpose(1, 0, 2)
    ).astype(bf)
    # additive causal mask for transposed-score diag blocks: kill s > t
    mask = (np.tril(np.ones((128, 128), np.float32), -1) * -1e5).astype(bf)
    idb = np.eye(128, dtype=np.float32).astype(bf)
    idf = np.eye(128, dtype=np.float32)
    in_maps = []
    for i in range(N_CORES):
        xTt = np.ascontiguousarray(
            x[i].T.reshape(N_CT, 128, T).transpose(1, 0, 2)
        ).astype(bf)
        in_maps.append(
            {"xTt": xTt, "wqkv": wqkv_tiled, "maskb": mask, "idb": idb,
             "idf": idf}
        )
    return in_maps


def run(x, w_q, w_k, w_v, trace=False, **trace_kwargs):
    from concourse.bass_utils import run_bass_kernel_spmd

    nc = _get_nc()
    in_maps = _host_inputs(x, w_q, w_k, w_v)
    res = run_bass_kernel_spmd(
        nc, in_maps, core_ids=list(range(N_CORES)), trace=trace, **trace_kwargs
    )
    outs = []
    for i in range(N_CORES):
        ot = np.asarray(res.results[i]["outt"])  # [128, 16, 64]
        outs.append(ot.transpose(1, 0, 2).reshape(T, H))
    return np.stack(outs).astype(np.float32), res


def kernel(x, w_q, w_k, w_v):
    out, _ = run(x, w_q, w_k, w_v, trace=False)
    return out


# revision 12
# speedup vs baseline: 1.2519x; 1.2519x over previous
"""Distributed Trainium2 kernel for a single attention head.

Problem: x:[8,2048,1024] f32, w_q/w_k/w_v:[1024,64] f32
  q,k,v = x@w ; scores = (q k^T)/sqrt(1024) causal-masked; out = softmax(scores)@v

Sharding: data-parallel over batch B=8 across the 8 NeuronCores (one batch
element per core, weights replicated, no collectives).

Per-core dataflow (T=2048, C=1024, H=64):
  - host ships x^T c-tiled [128, 8, T] bf16 and packed wqkv [128, 8, 192]
    bf16; output is written tiled [128, 16, 64] f32 and re-permuted on host.
  - projections: weights stationary, K=128 c-tiles (PSUM accumulation caps
    at 128 contraction elems/cycle, so row-packing buys nothing here);
    qk packed in one pass (M=128), v in a second (M=64).
  - scores per s-tile pair run concurrently in PE row groups 0/1 (single-
    shot K=64 matmuls reach 2 cols/cycle); q/k duplicated onto both
    partition halves via SBUF DMA to enable the packing.
  - causal diag blocks: -1e5 additive mask via identity matmul on the PE
    (keeps the scores->exp->PV chain on two engines only).
  - exp on ScalarE with scale=1/32 folded in, fully-masked column prefix
    trimmed; output bf16.
  - PV: out^T[h,t] accumulated serially over s-tiles with lhsT = [v | 1] so
    row 64 is the softmax denominator (fused row-sum).
  - epilogue: f32 PE transpose back to [t,h], reciprocal-scale on VectorE,
    one tiled output DMA per chunk; acc is double-buffered so the next
    chunk's PV does not wait on the epilogue copy.
  - x input: chunk 0 c-tiles split across the 3 DMA queues for latency;
    chunks 1-3 as half-chunk strided DMAs; next-chunk projection emission
    interleaved between score pairs keeps the PE queue dense.
"""

import os
import sys

import numpy as np

for p in ("/opt/trn_rl_repo",):
    if p not in sys.path and os.path.isdir(p):
        sys.path.insert(0, p)

import ml_dtypes  # noqa: E402

B, T, C, H = 8, 2048, 1024, 64
N_CORES = 8
TCH = 512                  # t-chunk (columns per PSUM bank of f32)
N_CHUNK = T // TCH         # 4
N_CT = C // 128            # 8 contraction tiles
SCALE = float(C) ** -0.5   # 1/32

_CACHE = {}


def _build():
    """Build + compile the SPMD Bass graph (same graph on all 8 cores)."""
    import concourse.bass as bass
    import concourse.mybir as mybir
    import concourse.tile as tile
    from concourse import bacc

    f32 = mybir.dt.float32
    bf16 = mybir.dt.bfloat16
    EXP = mybir.ActivationFunctionType.Exp

    nc = bacc.Bacc(
        "TRN2", target_bir_lowering=False, debug=False, num_devices=N_CORES
    )

    xT_d = nc.dram_tensor("xTt", [128, N_CT, T], bf16, kind="ExternalInput")
    wqkv_d = nc.dram_tensor("wqkv", [128, N_CT, 192], bf16, kind="ExternalInput")
    mask_d = nc.dram_tensor("maskb", [128, 128], bf16, kind="ExternalInput")
    idb_d = nc.dram_tensor("idb", [128, 128], bf16, kind="ExternalInput")
    out_d = nc.dram_tensor("outt", [128, T // 128, H], f32, kind="ExternalOutput")

    with tile.TileContext(nc) as tc:
        with (
            tc.tile_pool(name="const", bufs=1) as constp,
            tc.tile_pool(name="xTp", bufs=1) as xTp,
            tc.tile_pool(name="qkp", bufs=1) as qkp,
            tc.tile_pool(name="v1p", bufs=1) as v1p,
            tc.tile_pool(name="exp", bufs=6) as expp,
            tc.tile_pool(name="epi", bufs=2) as epip,
            tc.tile_pool(name="outb", bufs=2) as outp,
            tc.tile_pool(name="Sp", bufs=2, space="PSUM") as Sp,
            tc.tile_pool(name="projp", bufs=2, space="PSUM") as projp,
            tc.tile_pool(name="accp", bufs=2, space="PSUM") as accp,
        ):
            # ---- weights: c-tile 0 first (unblocks first matmul)
            wqkv_t = constp.tile([128, N_CT, 192], bf16, tag="wqkv", name="wqkv_t")
            nc.sync.dma_start(out=wqkv_t[:, 0, :], in_=wqkv_d[:, 0, :])

            mask_t = constp.tile([128, 128], bf16, tag="mask", name="mask_t")
            nc.gpsimd.dma_start(out=mask_t[:], in_=mask_d[:])
            idb_t = constp.tile([128, 128], bf16, tag="idb", name="idb_t")
            nc.gpsimd.dma_start(out=idb_t[:], in_=idb_d[:])

            # ---- x^T input feed (3 DMA queues: sync / scalar / gpsimd)
            xt = {}
            for t in range(N_CHUNK):
                xt[t] = xTp.tile([128, N_CT, TCH], bf16, tag=f"x{t}", name=f"x{t}")
            nc.sync.dma_start(out=xt[0][:, 0:1, :], in_=xT_d[:, 0:1, 0:TCH])
            nc.scalar.dma_start(out=xt[0][:, 1:3, :], in_=xT_d[:, 1:3, 0:TCH])
            nc.gpsimd.dma_start(out=xt[0][:, 3:5, :], in_=xT_d[:, 3:5, 0:TCH])
            nc.sync.dma_start(out=xt[0][:, 5:8, :], in_=xT_d[:, 5:8, 0:TCH])
            nc.scalar.dma_start(out=wqkv_t[:, 1:, :], in_=wqkv_d[:, 1:, :])
            nc.gpsimd.dma_start(
                out=xt[1][:, 0:4, :], in_=xT_d[:, 0:4, TCH : 2 * TCH]
            )
            nc.scalar.dma_start(
                out=xt[1][:, 4:8, :], in_=xT_d[:, 4:8, TCH : 2 * TCH]
            )
            nc.scalar.dma_start(
                out=xt[2][:, 0:4, :], in_=xT_d[:, 0:4, 2 * TCH : 3 * TCH]
            )
            nc.gpsimd.dma_start(
                out=xt[2][:, 4:8, :], in_=xT_d[:, 4:8, 2 * TCH : 3 * TCH]
            )
            nc.gpsimd.dma_start(out=xt[3][:, 0:4, :], in_=xT_d[:, 0:4, 3 * TCH :])
            nc.scalar.dma_start(out=xt[3][:, 4:8, :], in_=xT_d[:, 4:8, 3 * TCH :])

            q2 = {}   # [128, TCH] bf16: qT duplicated on both partition halves
            k2 = {}   # [128, TCH] bf16: kT duplicated on both partition halves
            v1 = {}   # [128, 65] bf16 per s-tile: [v | 1]

            def proj_steps(tch):
                """Emission thunks for chunk `tch`'s projections."""
                qk_steps = []
                v_steps = []
                st = {}

                def qk_mm(c):
                    def f():
                        if c == 0:
                            st["S"] = projp.tile(
                                [128, TCH], f32, tag="pj", name=f"Sqk{tch}"
                            )
                        nc.tensor.matmul(
                            st["S"][:],
                            wqkv_t[:, c, 0:128],
                            xt[tch][:, c, :],
                            start=(c == 0),
                            stop=(c == N_CT - 1),
                            skip_group_check=True,
                        )
                    return f

                def qk_out():
                    qt = qkp.tile([128, TCH], bf16, tag=f"q2_{tch}", name=f"q2_{tch}")
                    kt = qkp.tile([128, TCH], bf16, tag=f"k2_{tch}", name=f"k2_{tch}")
                    nc.vector.tensor_copy(qt[0:64, :], st["S"][0:64, :])
                    nc.vector.tensor_copy(kt[64:128, :], st["S"][64:128, :])
                    nc.sync.dma_start(out=qt[64:128, :], in_=qt[0:64, :])
                    nc.sync.dma_start(out=kt[0:64, :], in_=kt[64:128, :])
                    q2[tch] = qt
                    k2[tch] = kt

                def v_mm(c):
                    def f():
                        if c == 0:
                            st["Pv"] = projp.tile(
                                [64, TCH], f32, tag="pj", name=f"Pv{tch}"
                            )
                        nc.tensor.matmul(
                            st["Pv"][:],
                            wqkv_t[:, c, 128:192],
                            xt[tch][:, c, :],
                            start=(c == 0),
                            stop=(c == N_CT - 1),
                            skip_group_check=True,
                        )
                    return f

                def v_out():
                    vTt = qkp.tile([64, TCH], bf16, tag=f"vT{tch}", name=f"vT{tch}")
                    nc.vector.tensor_copy(vTt[:], st["Pv"][:])
                    st["vT"] = vTt

                def v1_build(i):
                    def f():
                        j = 4 * tch + i
                        Pt = projp.tile([128, 64], bf16, tag="pj", name=f"Pt{j}")
                        nc.tensor.transpose(
                            Pt[:],
                            st["vT"][:, 128 * i : 128 * (i + 1)],
                            idb_t[0:64, 0:64],
                        )
                        v1t = v1p.tile([128, 65], bf16, tag=f"v1_{j}", name=f"v1_{j}")
                        nc.vector.tensor_copy(v1t[:, 0:64], Pt[:])
                        nc.vector.memset(v1t[:, 64:65], 1.0)
                        v1[j] = v1t
                    return f

                for c in range(N_CT):
                    qk_steps.append(qk_mm(c))
                qk_steps.append(qk_out)
                for c in range(N_CT):
                    v_steps.append(v_mm(c))
                v_steps.append(v_out)
                for i in range(4):
                    v_steps.append(v1_build(i))
                return qk_steps, v_steps

            # chunk-0 qk projection up front (v after the first scores pair)
            qk0, v0 = proj_steps(0)
            for s in qk0:
                s()

            for tch in range(N_CHUNK):
                if tch + 1 < N_CHUNK:
                    qkn, vn = proj_steps(tch + 1)
                    pending = qkn + vn
                else:
                    pending = []
                jmax = 4 * tch + 3
                pairs = list(range(0, jmax + 1, 2))
                per_pair = -(-len(pending) // len(pairs)) if pending else 0

                acc = accp.tile([65, TCH], f32, tag="acc", name=f"acc{tch}")
                for pi, jp in enumerate(pairs):
                    S2 = Sp.tile([128, 2, TCH], f32, tag="S", name=f"S{tch}_{jp}")
                    for jj in range(2):
                        j = jp + jj
                        half = slice(64 * jj, 64 * (jj + 1))
                        ksl = k2[j // 4][half, 128 * (j % 4) : 128 * (j % 4 + 1)]
                        rel = j - 4 * tch
                        nc.tensor.matmul(
                            S2[:, jj, :],
                            ksl,
                            q2[tch][half, :],
                            start=True,
                            stop=(rel < 0),
                            skip_group_check=True,
                        )
                        if rel >= 0:
                            # additive -1e5 strict-lower-tri mask into the
                            # diagonal block via identity matmul (PE-only
                            # chain keeps exp off the vector engine)
                            a = 128 * rel
                            nc.tensor.matmul(
                                S2[:, jj, a : a + 128],
                                idb_t[:],
                                mask_t[:],
                                start=False,
                                stop=True,
                                skip_group_check=True,
                            )
                    # exp; skip the fully-masked column prefix of the pair
                    lo0 = 128 * max(0, jp - 4 * tch)
                    ext = expp.tile(
                        [128, 2, TCH], bf16, tag="ex", name=f"ex{tch}_{jp}"
                    )
                    nc.scalar.activation(
                        ext[:, :, lo0:], S2[:, :, lo0:], EXP, scale=SCALE
                    )
                    if tch == 0 and pi == 0:
                        for s in v0:
                            s()
                    # PV accumulation (ones col adds the softmax denominator)
                    for jj in range(2):
                        j = jp + jj
                        lo = 128 * max(0, j - 4 * tch)
                        nc.tensor.matmul(
                            acc[:, lo:TCH] if j > 0 else acc[:, :],
                            v1[j][:],
                            ext[:, jj, lo:TCH],
                            start=(j == 0),
                            stop=(j == jmax),
                            skip_group_check=True,
                        )
                    for _ in range(per_pair):
                        if pending:
                            pending.pop(0)()
                for s in pending:
                    s()

                # ======== epilogue: normalize + transpose + DMA out ========
                oT = epip.tile([65, TCH], bf16, tag="oT", name=f"oT{tch}")
                nc.vector.tensor_copy(oT[:], acc[:])
                ob = outp.tile([128, 4, H], f32, tag="ob", name=f"ob{tch}")
                for i in range(4):
                    Pe = projp.tile([128, 65], bf16, tag="pj", name=f"Pe{tch}_{i}")
                    nc.tensor.transpose(
                        Pe[:],
                        oT[:, 128 * i : 128 * (i + 1)],
                        idb_t[0:65, 0:65],
                    )
                    rec = epip.tile([128, 1], f32, tag="rec", name=f"rec{tch}_{i}")
                    nc.vector.reciprocal(rec[:], Pe[:, 64:65])
                    nc.vector.tensor_scalar_mul(ob[:, i, :], Pe[:, 0:64], rec[:])
                nc.sync.dma_start(
                    out=out_d[:, 4 * tch : 4 * tch + 4, :], in_=ob[:]
                )

    nc.compile()
    return nc


def _get_nc():
    if "nc" not in _CACHE:
        _CACHE["nc"] = _build()
    return _CACHE["nc"]


def _host_inputs(x, w_q, w_k, w_v):
    bf = ml_dtypes.bfloat16
    x = np.asarray(x, dtype=np.float32)
    wqkv = np.concatenate(
        [np.asarray(w_q, np.float32), np.asarray(w_k, np.float32),
         np.asarray(w_v, np.float32)], 1
    )
    wqkv_tiled = np.ascontiguousarray(
        wqkv.reshape(N_CT, 128, 192).transpose(1, 0, 2)
    ).astype(bf)
    # additive causal mask for transposed-score diag blocks: kill s > t
    mask = (np.tril(np.ones((128, 128), np.float32), -1) * -1e5).astype(bf)
    idb = np.eye(128, dtype=np.float32).astype(bf)
    in_maps = []
    for i in range(N_CORES):
        xTt = np.ascontiguousarray(
            x[i].T.reshape(N_CT, 128, T).transpose(1, 0, 2)
        ).astype(bf)
        in_maps.append(
            {"xTt": xTt, "wqkv": wqkv_tiled, "maskb": mask, "idb": idb}
        )
    return in_maps


def run(x, w_q, w_k, w_v, trace=False, **trace_kwargs):
    from concourse.bass_utils import run_bass_kernel_spmd

    nc = _get_nc()
    in_maps = _host_inputs(x, w_q, w_k, w_v)
    res = run_bass_kernel_spmd(
        nc, in_maps, core_ids=list(range(N_CORES)), trace=trace, **trace_kwargs
    )
    outs = []
    for i in range(N_CORES):
        ot = np.asarray(res.results[i]["outt"])  # [128, 16, 64]
        outs.append(ot.transpose(1, 0, 2).reshape(T, H))
    return np.stack(outs).astype(np.float32), res


def kernel(x, w_q, w_k, w_v):
    out, _ = run(x, w_q, w_k, w_v, trace=False)
    return out
